# revision 20
# baseline (speedup 1.0000x reference)
"""Trainium2 Bass kernel for the 3-body Hamiltonian-NN time-derivative.

out = J grad_z H(z):  dqdt = p * minv;  dpdt from pairwise forces.

The potential's tiny MLP acts on a scalar pairwise inverse distance, so its
gradient is a smooth 1-D function g(s).  The per-pair force coefficient
C(s) = g(s)*s^3 (s = 1/sqrt(r2+eps2)) is fit at runtime from the MLP weights
as s*poly_5(s) (weighted Chebyshev-style LSQ, force-weighted), turning the
kernel into cheap elementwise work, data-parallel over 8 NeuronCores.

Perf design (memory-regime problem; v1 was 135 us, f32 strided):
 - q ships as int16 fixed-point (adaptive scale QS, calibrated so
   max|dif|*QS < 32700): pair differences are exact in int16 and the descale
   folds free into ACT Square (scale=1/QS) and a second Exp (bias=-ln QS).
   fp16 q would fail the error budget; int16 fixed point passes 3.3x.
 - Fast path (minv == 1 to 1e-6, i.e. log_m_body = 0): dqdt = p exactly
   (error <= 1e-6 * |p|, 5 orders below budget), assembled host-side from the
   f32 input; the device only reads q (18 B/row) and writes dpdt fp16
   (18 B/row).  General-minv path ships p and scales on device.
 - Host marshals to a field-major layout ([...,128p, field, t], t contiguous)
   so every DVE op is step-1/4B-aligned: fp16 tensor_tensor hits the 2x DVE
   perf mode, immediate-scalar tensor_scalar hits 4x.
 - Poly evaluated in a nested even/odd form: 3 tensor_scalar (4x) for the
   (c_odd + c_even*s) pairs + 5 tensor_tensor (2x) combines, instead of
   Horner scalar_tensor_tensor steps (AP-scalar STT has NO fast mode: 1x).
 - All ACT functions (Square/Ln/Exp/Copy) come from the single
   natural_log_exp_and_others table set; a Bacc subclass pins the set-chooser
   to it so the per-iteration ACT table reloads (5 x ~1.3us greedy default)
   collapse to one hoisted load.
 - Input DMA on the sync HWDGE ring, output DMA on the scalar ring.
Validated end-to-end in a bit-faithful numpy simulation: abs err 0.033 vs
budget 0.107 (= 2e-2 * absmax(ref)), 3.3x margin.
"""
from contextlib import ExitStack

import numpy as np

import concourse.bass as bass
import concourse.tile as tile
from concourse import bacc, mybir
from concourse.bass_utils import run_bass_kernel_spmd
from concourse.hw_specs import get_activation_tables

F32 = mybir.dt.float32
F16 = mybir.dt.float16
I16 = mybir.dt.int16
AF = mybir.ActivationFunctionType
ALU = mybir.AluOpType

EPS2 = 0.01
XLO = float(np.log(EPS2))
XHI = float(np.log(130.0))
# Pair orientation (0,1),(1,2),(2,0) makes every dpdt output a plain
# two-operand subtraction of the per-pair forces (no negation op needed),
# and lets pair 0 start from just bodies {0,1} (split input DMA).
PAIRS = [(0, 1), (1, 2), (2, 0)]
NCORES = 8
DEG = 5               # C = s * poly_DEG(s); DEG=5 -> 6 coefficients
P = 128
TC = 256
NCHUNK = 1            # P * TC * NCHUNK rows per core
# Two independent pair-streams pipeline DVE against ACT through the
# dd->r2->ln->exp->poly chain at full FD (measured ~equal to 2 t-chunks,
# with half the per-iteration DMA fixed costs).
STREAMS = [(0, 1), (1, 3)]
ACT_SET = "natural_log_exp_and_others"


class _Bacc(bacc.Bacc):
    """Bacc whose ACT-table chooser only sees ACT_SET (which contains every
    function this kernel uses: ln, exp, square, copy, identity).  The default
    chooser greedily picks the first set per function, alternating between
    exp_and_others and natural_log and re-loading tables 5x per pass."""

    def insert_act_table_loads(self):
        has_activation = any(
            isinstance(i, mybir.InstActivation)
            for b in self.main_func.blocks
            for i in b.instructions
        )
        if not has_activation:
            return
        tables = [
            (name, (funcs if name == ACT_SET else set()))
            for name, funcs in get_activation_tables(self.m.arch).items()
        ]
        bacc._bass_rust.insert_act_table_loads(self, tables)


def _silu(x):
    return x / (1.0 + np.exp(-x))


def _dsilu(x):
    sg = 1.0 / (1.0 + np.exp(-x))
    return sg * (1.0 + x * (1.0 - sg))


def _fit_force_poly_s(W1, b1, W2, b2, W3, deg=DEG):
    """Coefficients c[0..deg] with C(s) = g(s)*s^3 ~= sum_k c[k] s^(k+1),
    least-squares weighted by 1/s so the force error |dC|*d is equi-ripple."""
    W1 = np.asarray(W1, np.float64); b1 = np.asarray(b1, np.float64)
    W2 = np.asarray(W2, np.float64); b2 = np.asarray(b2, np.float64)
    W3 = np.asarray(W3, np.float64)

    def g_exact(s):
        s = np.asarray(s, np.float64)[..., None]
        u1 = s * W1[:, 0] + b1
        a1 = _silu(u1)
        u2 = a1 @ W2.T + b2
        d2 = W3[0] * _dsilu(u2)
        d1 = (d2 @ W2) * _dsilu(u1)
        return d1 @ W1[:, 0]

    n = 6000
    xk = np.cos(np.pi * (np.arange(n) + 0.5) / n)
    xs = XLO + (xk + 1) * (XHI - XLO) / 2
    s = np.exp(-0.5 * xs)
    C = g_exact(s) * s ** 3
    A = s[:, None] ** np.arange(1, deg + 2)[None, :]
    w = (1.0 / s)[:, None]
    coef, *_ = np.linalg.lstsq(A * w, C * w[:, 0], rcond=None)
    return coef


def _emit_consts(tc, ctx, qs, minv_ap=None):
    """One-time constant tiles (own pool; callers may hoist out of a
    timing repeat-loop since the real kernel pays this exactly once)."""
    nc = tc.nc
    const = ctx.enter_context(tc.tile_pool(name="const", bufs=1))
    eps_t = const.tile([P, 1], F32)
    nc.vector.memset(eps_t[:], EPS2)
    lnqs_t = const.tile([P, 1], F32)
    nc.vector.memset(lnqs_t[:], -float(np.log(qs)))
    minv_t = None
    if minv_ap is not None:
        minv_t = const.tile([P, 3], F32)
        nc.sync.dma_start(minv_t[:], minv_ap)
    return eps_t, lnqs_t, minv_t


def _emit(tc, zq_ap, outp_ap, coefs, qs, nchunk, Tc,
          zp_ap=None, outq_ap=None, minv_ap=None, consts=None,
          streams=None):
    """Per-core body.  zq: [nchunk*P, 9*Tc] int16 (q * QS, field-major);
    outp: dpdt fp16 same layout.  zp/outq/minv only on the general-minv
    path (dqdt = p * minv on device)."""
    nc = tc.nc
    c = [float(v) for v in coefs]   # c[k] multiplies s^(k+1)
    zqr = zq_ap.rearrange("(n p) c -> n p c", p=P)
    opr = outp_ap.rearrange("(n p) c -> n p c", p=P)
    general = zp_ap is not None
    if general:
        zpr = zp_ap.rearrange("(n p) c -> n p c", p=P)
        oqr = outq_ap.rearrange("(n p) c -> n p c", p=P)

    if streams is None:
        streams = STREAMS
    with ExitStack() as ctx:
        if consts is None:
            consts = _emit_consts(tc, ctx, qs, minv_ap)
        eps_t, lnqs_t, minv_t = consts
        iop = ctx.enter_context(tc.tile_pool(name="io", bufs=3))
        wk = ctx.enter_context(tc.tile_pool(name="wk", bufs=3))

        for ci in range(nchunk):
            zq = iop.tile([P, 9 * Tc], I16, tag="zq")
            # split input DMA: bodies {0,1} land first so pair (0,1) and its
            # whole stream run while body 2 is still in flight
            nc.sync.dma_start(zq[:, :6 * Tc], zqr[ci][:, :6 * Tc])
            nc.sync.dma_start(zq[:, 6 * Tc:], zqr[ci][:, 6 * Tc:])
            if general:
                zp = iop.tile([P, 9 * Tc], F16, tag="zp")
                nc.sync.dma_start(zp[:], zpr[ci])

            dif = wk.tile([P, 9 * Tc], F16, tag="dif")
            fv = wk.tile([P, 9 * Tc], F16, tag="f")
            op_t = wk.tile([P, 9 * Tc], F16, tag="op")

            zq3 = zq[:].rearrange("p (b t) -> p b t", b=9)
            dif4 = dif[:].rearrange("p (k c t) -> p k c t", k=3, c=3)
            # pair (0,1) needs only the first input DMA; (1,2) and (2,0)
            # wait for body 2
            nc.vector.tensor_sub(dif4[:, 0], zq3[:, 0:3], zq3[:, 3:6])
            nc.vector.tensor_sub(dif4[:, 1], zq3[:, 3:6], zq3[:, 6:9])
            nc.vector.tensor_sub(dif4[:, 2], zq3[:, 6:9], zq3[:, 0:3])

            f4 = fv[:].rearrange("p (k c t) -> p k c t", k=3, c=3)
            for g, (k0, k1) in enumerate(streams):
                nk = k1 - k0
                W = nk * Tc
                dd = wk.tile([P, 3 * W], F16, tag=f"dd{g}")
                r2 = wk.tile([P, W], F16, tag=f"r2{g}")
                x = wk.tile([P, W], F32, tag=f"x{g}")
                s = wk.tile([P, W], F16, tag=f"s{g}")
                sq = wk.tile([P, W], F16, tag=f"sq{g}")
                sq2 = wk.tile([P, W], F16, tag=f"sq2{g}")
                sq4 = wk.tile([P, W], F16, tag=f"sq4{g}")
                At = wk.tile([P, W], F16, tag=f"At{g}")
                Bt = wk.tile([P, W], F16, tag=f"Bt{g}")
                Ht = wk.tile([P, W], F16, tag=f"Ht{g}")
                Cp = wk.tile([P, W], F16, tag=f"Cp{g}")

                # dd = (dif/QS)^2  (descale folded into the ACT affine)
                nc.scalar.activation(dd[:], dif[:, 3 * Tc * k0:3 * Tc * k1],
                                     AF.Square, scale=1.0 / qs)
                dd4 = dd[:].rearrange("p (k c t) -> p k c t", k=nk, c=3)
                r2v = r2[:].rearrange("p (k t) -> p k t", k=nk)
                nc.vector.tensor_add(r2v, dd4[:, :, 0], dd4[:, :, 1])
                nc.vector.tensor_add(r2v, r2v, dd4[:, :, 2])

                # x = ln(r2+eps2) in f32; fit domain [ln eps2, ln 130] covers
                # all reachable r2 for randn inputs (observed max 55.2)
                nc.scalar.activation(x[:], r2[:], AF.Ln, bias=eps_t[:, 0:1])
                nc.scalar.activation(s[:], x[:], AF.Exp, scale=-0.5)
                # Power-folded evaluation: sqN = s^N/QS comes straight from x
                # via extra Exp ops (ACT is dtype-independent 1x and has
                # slack); keeps the poly in real units (pre-scaled
                # coefficients go subnormal) and cuts the DVE dependency
                # chain from 5 serial combines (Horner in s^2) to 3.
                nc.scalar.activation(sq[:], x[:], AF.Exp, scale=-0.5,
                                     bias=lnqs_t[:, 0:1])
                nc.scalar.activation(sq2[:], x[:], AF.Exp, scale=-1.5,
                                     bias=lnqs_t[:, 0:1])
                nc.scalar.activation(sq4[:], x[:], AF.Exp, scale=-2.5,
                                     bias=lnqs_t[:, 0:1])

                # C/QS = (c0+c1 s)*sq + (c2+c3 s)*sq2 + (c4+c5 s)*sq4
                # (tensor_scalar pairs run at 4x, tensor_tensor at 2x;
                # AP-scalar STT would be 1x)
                nc.vector.tensor_scalar(At[:], s[:], c[1], c[0], ALU.mult, ALU.add)
                nc.vector.tensor_scalar(Bt[:], s[:], c[3], c[2], ALU.mult, ALU.add)
                nc.vector.tensor_scalar(Ht[:], s[:], c[5], c[4], ALU.mult, ALU.add)
                nc.vector.tensor_mul(At[:], At[:], sq[:])
                nc.vector.tensor_mul(Bt[:], Bt[:], sq2[:])
                nc.vector.tensor_mul(Ht[:], Ht[:], sq4[:])
                nc.vector.tensor_add(At[:], At[:], Bt[:])
                nc.vector.tensor_add(Cp[:], At[:], Ht[:])

                Cv = Cp[:].rearrange("p (k t) -> p k t", k=nk)
                for cdim in range(3):
                    nc.vector.tensor_mul(f4[:, k0:k1, cdim],
                                         dif4[:, k0:k1, cdim], Cv)

            # dpdt: with pair orientation (0,1),(1,2),(2,0):
            #   b0 = f0 - f2, b1 = f1 - f0, b2 = f2 - f1
            # Each body-block DMAs out as soon as its subtraction lands, so
            # the store drains while the remaining combines still run.
            opb = op_t[:].rearrange("p (b w) -> p b w", b=3)
            f3 = fv[:].rearrange("p (k w) -> p k w", k=3)
            opr3 = opr[ci].rearrange("p (b w) -> p b w", b=3)
            nc.vector.tensor_sub(opb[:, 0], f3[:, 0], f3[:, 2])
            nc.scalar.dma_start(opr3[:, 0], opb[:, 0])
            nc.vector.tensor_sub(opb[:, 1], f3[:, 1], f3[:, 0])
            nc.scalar.dma_start(opr3[:, 1], opb[:, 1])
            nc.vector.tensor_sub(opb[:, 2], f3[:, 2], f3[:, 1])
            nc.scalar.dma_start(opr3[:, 2], opb[:, 2])

            if general:
                zpb = zp[:].rearrange("p (b w) -> p b w", b=3)
                for b in range(3):
                    nc.scalar.activation(zpb[:, b], zpb[:, b], AF.Copy,
                                         scale=minv_t[:, b:b + 1])
                nc.scalar.dma_start(oqr[ci], zp[:])


_MODULE_CACHE = {}


def _get_module(coefs, qs, nchunk=NCHUNK, Tc=TC, fast=True, streams=None):
    key = (tuple(np.round(coefs, 12)), qs, nchunk, Tc, fast,
           tuple(streams or ()))
    if key not in _MODULE_CACHE:
        nc = _Bacc("TRN2", target_bir_lowering=False, debug=False,
                   num_devices=NCORES)
        rows = nchunk * P
        zq = nc.dram_tensor("zq", [rows, 9 * Tc], I16, kind="ExternalInput").ap()
        outp = nc.dram_tensor("outp", [rows, 9 * Tc], F16,
                              kind="ExternalOutput").ap()
        kw = {}
        if not fast:
            kw["zp_ap"] = nc.dram_tensor("zp", [rows, 9 * Tc], F16,
                                         kind="ExternalInput").ap()
            kw["minv_ap"] = nc.dram_tensor("minv", [P, 3], F32,
                                           kind="ExternalInput").ap()
            kw["outq_ap"] = nc.dram_tensor("outq", [rows, 9 * Tc], F16,
                                           kind="ExternalOutput").ap()
        with tile.TileContext(nc) as tc:
            _emit(tc, zq, outp, coefs, qs, nchunk, Tc, streams=streams, **kw)
        nc.compile()
        _MODULE_CACHE[key] = nc
    return _MODULE_CACHE[key]


def _marshal(arr, nchunk, Tc):
    """[B_core rows, 9] -> [nchunk*P, 9*Tc] field-major per chunk."""
    return np.ascontiguousarray(
        arr.reshape(nchunk, P, Tc, 9).transpose(0, 1, 3, 2)
        .reshape(nchunk * P, 9 * Tc))


def _unmarshal(arr, nchunk, Tc):
    return arr.reshape(nchunk, P, 9, Tc).transpose(0, 1, 3, 2).reshape(-1, 9)


def host_inputs(z, log_m_body, W1, b1, W2, b2, W3, nchunk=NCHUNK, Tc=TC,
                deg=DEG):
    """Returns (in_maps, coefs, qs, fast, B_pad)."""
    z = np.asarray(z, np.float32)
    B = z.shape[0]
    grain = NCORES * P * Tc * nchunk
    B_pad = ((B + grain - 1) // grain) * grain
    if B_pad != B:
        zp_ = np.zeros((B_pad, 18), np.float32)
        zp_[:B] = z
        z = zp_

    coefs = _fit_force_poly_s(W1, b1, W2, b2, W3, deg=deg)
    minv = (np.float64(1.0)
            / (np.exp(np.asarray(log_m_body, np.float64)) + 1e-8))
    fast = bool(np.abs(minv - 1.0).max() < 1e-6)
    minv128 = np.ascontiguousarray(
        np.tile(minv[None, :].astype(np.float32), (P, 1)))

    # adaptive fixed-point scale: largest power of 2 with max|dif|*QS < 32700
    q = z[:, :9]
    qb = q.reshape(-1, 3, 3)
    difmax = max(
        float(np.abs(qb[:, i] - qb[:, j]).max()) for i, j in PAIRS)
    qs = float(2.0 ** np.floor(np.log2(32700.0 / max(difmax, 1e-3))))
    qs = float(min(max(qs, 256.0), 16384.0))

    qi = np.clip(np.round(q.astype(np.float64) * qs),
                 -32767, 32767).astype(np.int16)
    p16 = z[:, 9:].astype(np.float16)

    B_core = B_pad // NCORES
    in_maps = []
    for i in range(NCORES):
        sl = slice(i * B_core, (i + 1) * B_core)
        m = {"zq": _marshal(qi[sl], nchunk, Tc)}
        if not fast:
            m["zp"] = _marshal(p16[sl], nchunk, Tc)
            m["minv"] = minv128
        in_maps.append(m)
    return in_maps, coefs, qs, fast, B_pad


def kernel(z, log_m_body, W1, b1, W2, b2, W3, b3, **_unused):
    z = np.asarray(z, np.float32)
    B = z.shape[0]
    in_maps, coefs, qs, fast, B_pad = host_inputs(
        z, log_m_body, W1, b1, W2, b2, W3)
    nc = _get_module(coefs, qs, fast=fast)
    res = run_bass_kernel_spmd(nc, in_maps, core_ids=list(range(NCORES)))
    dps = [_unmarshal(np.asarray(r["outp"]), NCHUNK, TC) for r in res.results]
    dpdt = np.concatenate(dps, axis=0).astype(np.float32)[:B]
    out = np.empty((B, 18), np.float32)
    if fast:
        # minv == 1 to <=1e-6: dqdt = p exactly (error 5 orders below budget)
        out[:, :9] = z[:B, 9:]
    else:
        dqs = [_unmarshal(np.asarray(r["outq"]), NCHUNK, TC)
               for r in res.results]
        out[:, :9] = np.concatenate(dqs, axis=0).astype(np.float32)[:B]
    out[:, 9:] = dpdt
    return out


# revision 21
# speedup vs baseline: 1.0361x; 1.0361x over previous
"""Trainium2 Bass kernel for the 3-body Hamiltonian-NN time-derivative.

out = J grad_z H(z):  dqdt = p * minv;  dpdt from pairwise forces.

The potential's tiny MLP acts on a scalar pairwise inverse distance, so its
gradient is a smooth 1-D function g(s).  The per-pair force coefficient
C(s) = g(s)*s^3 (s = 1/sqrt(r2+eps2)) is fit at runtime from the MLP weights
as s*poly_5(s) (weighted Chebyshev-style LSQ, force-weighted), turning the
kernel into cheap elementwise work, data-parallel over 8 NeuronCores.

Perf design (memory-regime problem; v1 was 135 us, f32 strided):
 - q ships as int16 fixed-point (adaptive scale QS, calibrated so
   max|dif|*QS < 32700): pair differences are exact in int16 and the descale
   folds free into ACT Square (scale=1/QS) and a second Exp (bias=-ln QS).
   fp16 q would fail the error budget; int16 fixed point passes 3.3x.
 - Fast path (minv == 1 to 1e-6, i.e. log_m_body = 0): dqdt = p exactly
   (error <= 1e-6 * |p|, 5 orders below budget), assembled host-side from the
   f32 input; the device only reads q (18 B/row) and writes dpdt fp16
   (18 B/row).  General-minv path ships p and scales on device.
 - Host marshals to a field-major layout ([...,128p, field, t], t contiguous)
   so every DVE op is step-1/4B-aligned: fp16 tensor_tensor hits the 2x DVE
   perf mode, immediate-scalar tensor_scalar hits 4x.
 - Poly evaluated in a nested even/odd form: 3 tensor_scalar (4x) for the
   (c_odd + c_even*s) pairs + 5 tensor_tensor (2x) combines, instead of
   Horner scalar_tensor_tensor steps (AP-scalar STT has NO fast mode: 1x).
 - All ACT functions (Square/Ln/Exp/Copy) come from the single
   natural_log_exp_and_others table set; a Bacc subclass pins the set-chooser
   to it so the per-iteration ACT table reloads (5 x ~1.3us greedy default)
   collapse to one hoisted load.
 - Input DMA on the sync HWDGE ring, output DMA on the scalar ring.
Validated end-to-end in a bit-faithful numpy simulation: abs err 0.033 vs
budget 0.107 (= 2e-2 * absmax(ref)), 3.3x margin.
"""
from contextlib import ExitStack

import numpy as np

import concourse.bass as bass
import concourse.tile as tile
from concourse import bacc, mybir
from concourse.bass_utils import run_bass_kernel_spmd
from concourse.hw_specs import get_activation_tables

F32 = mybir.dt.float32
F16 = mybir.dt.float16
I16 = mybir.dt.int16
AF = mybir.ActivationFunctionType
ALU = mybir.AluOpType

EPS2 = 0.01
XLO = float(np.log(EPS2))
XHI = float(np.log(130.0))
# Pair orientation (0,1),(2,0),(1,2) makes every dpdt output a plain
# two-operand subtraction of the per-pair forces (no negation op needed).
PAIRS = [(0, 1), (2, 0), (1, 2)]
NCORES = 8
DEG = 5               # C = s * poly_DEG(s); DEG=5 -> 6 coefficients
P = 128
TC = 256
NCHUNK = 1            # P * TC * NCHUNK rows per core
# Two independent pair-streams pipeline DVE against ACT through the
# dd->r2->ln->exp->poly chain at full FD (measured ~equal to 2 t-chunks,
# with half the per-iteration DMA fixed costs).
STREAMS = [(0, 2), (2, 3)]
ACT_SET = "natural_log_exp_and_others"


class _Bacc(bacc.Bacc):
    """Bacc whose ACT-table chooser only sees ACT_SET (which contains every
    function this kernel uses: ln, exp, square, copy, identity).  The default
    chooser greedily picks the first set per function, alternating between
    exp_and_others and natural_log and re-loading tables 5x per pass."""

    def insert_act_table_loads(self):
        has_activation = any(
            isinstance(i, mybir.InstActivation)
            for b in self.main_func.blocks
            for i in b.instructions
        )
        if not has_activation:
            return
        tables = [
            (name, (funcs if name == ACT_SET else set()))
            for name, funcs in get_activation_tables(self.m.arch).items()
        ]
        bacc._bass_rust.insert_act_table_loads(self, tables)


def _silu(x):
    return x / (1.0 + np.exp(-x))


def _dsilu(x):
    sg = 1.0 / (1.0 + np.exp(-x))
    return sg * (1.0 + x * (1.0 - sg))


def _fit_force_poly_s(W1, b1, W2, b2, W3, deg=DEG):
    """Coefficients c[0..deg] with C(s) = g(s)*s^3 ~= sum_k c[k] s^(k+1),
    least-squares weighted by 1/s so the force error |dC|*d is equi-ripple."""
    W1 = np.asarray(W1, np.float64); b1 = np.asarray(b1, np.float64)
    W2 = np.asarray(W2, np.float64); b2 = np.asarray(b2, np.float64)
    W3 = np.asarray(W3, np.float64)

    def g_exact(s):
        s = np.asarray(s, np.float64)[..., None]
        u1 = s * W1[:, 0] + b1
        a1 = _silu(u1)
        u2 = a1 @ W2.T + b2
        d2 = W3[0] * _dsilu(u2)
        d1 = (d2 @ W2) * _dsilu(u1)
        return d1 @ W1[:, 0]

    n = 6000
    xk = np.cos(np.pi * (np.arange(n) + 0.5) / n)
    xs = XLO + (xk + 1) * (XHI - XLO) / 2
    s = np.exp(-0.5 * xs)
    C = g_exact(s) * s ** 3
    A = s[:, None] ** np.arange(1, deg + 2)[None, :]
    w = (1.0 / s)[:, None]
    coef, *_ = np.linalg.lstsq(A * w, C * w[:, 0], rcond=None)
    return coef


def _emit_consts(tc, ctx, qs, minv_ap=None):
    """One-time constant tiles (own pool; callers may hoist out of a
    timing repeat-loop since the real kernel pays this exactly once)."""
    nc = tc.nc
    const = ctx.enter_context(tc.tile_pool(name="const", bufs=1))
    eps_t = const.tile([P, 1], F32)
    nc.vector.memset(eps_t[:], EPS2)
    lnqs_t = const.tile([P, 1], F32)
    nc.vector.memset(lnqs_t[:], -float(np.log(qs)))
    minv_t = None
    if minv_ap is not None:
        minv_t = const.tile([P, 3], F32)
        nc.sync.dma_start(minv_t[:], minv_ap)
    return eps_t, lnqs_t, minv_t


def _emit(tc, zq_ap, outp_ap, coefs, qs, nchunk, Tc,
          zp_ap=None, outq_ap=None, minv_ap=None, consts=None,
          streams=None):
    """Per-core body.  zq: [nchunk*P, 9*Tc] int16 (q * QS, field-major);
    outp: dpdt fp16 same layout.  zp/outq/minv only on the general-minv
    path (dqdt = p * minv on device)."""
    nc = tc.nc
    c = [float(v) for v in coefs]   # c[k] multiplies s^(k+1)
    zqr = zq_ap.rearrange("(n p) c -> n p c", p=P)
    opr = outp_ap.rearrange("(n p) c -> n p c", p=P)
    general = zp_ap is not None
    if general:
        zpr = zp_ap.rearrange("(n p) c -> n p c", p=P)
        oqr = outq_ap.rearrange("(n p) c -> n p c", p=P)

    if streams is None:
        streams = STREAMS
    with ExitStack() as ctx:
        if consts is None:
            consts = _emit_consts(tc, ctx, qs, minv_ap)
        eps_t, lnqs_t, minv_t = consts
        iop = ctx.enter_context(tc.tile_pool(name="io", bufs=3))
        wk = ctx.enter_context(tc.tile_pool(name="wk", bufs=3))

        for ci in range(nchunk):
            zq = iop.tile([P, 9 * Tc], I16, tag="zq")
            nc.sync.dma_start(zq[:], zqr[ci])
            if general:
                zp = iop.tile([P, 9 * Tc], F16, tag="zp")
                nc.sync.dma_start(zp[:], zpr[ci])

            dif = wk.tile([P, 9 * Tc], F16, tag="dif")
            fv = wk.tile([P, 9 * Tc], F16, tag="f")
            op_t = wk.tile([P, 9 * Tc], F16, tag="op")

            zq3 = zq[:].rearrange("p (b t) -> p b t", b=9)
            dif4 = dif[:].rearrange("p (k c t) -> p k c t", k=3, c=3)
            # pairs 0=(0,1) and 2=(1,2) share the affine form (k, k+1):
            # one op covers both (dif blocks 0 and 2, stride-2 k-slice)
            nc.vector.tensor_sub(
                dif4[:, 0:3:2], zq3[:, 0:6].rearrange("p (k c) t -> p k c t", k=2),
                zq3[:, 3:9].rearrange("p (k c) t -> p k c t", k=2))
            nc.vector.tensor_sub(dif4[:, 1], zq3[:, 6:9], zq3[:, 0:3])

            f4 = fv[:].rearrange("p (k c t) -> p k c t", k=3, c=3)
            for g, (k0, k1) in enumerate(streams):
                nk = k1 - k0
                W = nk * Tc
                dd = wk.tile([P, 3 * W], F16, tag=f"dd{g}")
                r2 = wk.tile([P, W], F16, tag=f"r2{g}")
                x = wk.tile([P, W], F32, tag=f"x{g}")
                s = wk.tile([P, W], F16, tag=f"s{g}")
                sq = wk.tile([P, W], F16, tag=f"sq{g}")
                sq2 = wk.tile([P, W], F16, tag=f"sq2{g}")
                sq4 = wk.tile([P, W], F16, tag=f"sq4{g}")
                At = wk.tile([P, W], F16, tag=f"At{g}")
                Bt = wk.tile([P, W], F16, tag=f"Bt{g}")
                Ht = wk.tile([P, W], F16, tag=f"Ht{g}")
                Cp = wk.tile([P, W], F16, tag=f"Cp{g}")

                # dd = (dif/QS)^2  (descale folded into the ACT affine)
                nc.scalar.activation(dd[:], dif[:, 3 * Tc * k0:3 * Tc * k1],
                                     AF.Square, scale=1.0 / qs)
                dd4 = dd[:].rearrange("p (k c t) -> p k c t", k=nk, c=3)
                r2v = r2[:].rearrange("p (k t) -> p k t", k=nk)
                nc.vector.tensor_add(r2v, dd4[:, :, 0], dd4[:, :, 1])
                nc.vector.tensor_add(r2v, r2v, dd4[:, :, 2])

                # x = ln(r2+eps2) in f32; fit domain [ln eps2, ln 130] covers
                # all reachable r2 for randn inputs (observed max 55.2)
                nc.scalar.activation(x[:], r2[:], AF.Ln, bias=eps_t[:, 0:1])
                nc.scalar.activation(s[:], x[:], AF.Exp, scale=-0.5)
                # Power-folded evaluation: sqN = s^N/QS comes straight from x
                # via extra Exp ops (ACT is dtype-independent 1x and has
                # slack); keeps the poly in real units (pre-scaled
                # coefficients go subnormal) and cuts the DVE dependency
                # chain from 5 serial combines (Horner in s^2) to 3.
                nc.scalar.activation(sq[:], x[:], AF.Exp, scale=-0.5,
                                     bias=lnqs_t[:, 0:1])
                nc.scalar.activation(sq2[:], x[:], AF.Exp, scale=-1.5,
                                     bias=lnqs_t[:, 0:1])
                nc.scalar.activation(sq4[:], x[:], AF.Exp, scale=-2.5,
                                     bias=lnqs_t[:, 0:1])

                # C/QS = (c0+c1 s)*sq + (c2+c3 s)*sq2 + (c4+c5 s)*sq4
                # (tensor_scalar pairs run at 4x, tensor_tensor at 2x;
                # AP-scalar STT would be 1x)
                nc.vector.tensor_scalar(At[:], s[:], c[1], c[0], ALU.mult, ALU.add)
                nc.vector.tensor_scalar(Bt[:], s[:], c[3], c[2], ALU.mult, ALU.add)
                nc.vector.tensor_scalar(Ht[:], s[:], c[5], c[4], ALU.mult, ALU.add)
                nc.vector.tensor_mul(At[:], At[:], sq[:])
                nc.vector.tensor_mul(Bt[:], Bt[:], sq2[:])
                nc.vector.tensor_mul(Ht[:], Ht[:], sq4[:])
                nc.vector.tensor_add(At[:], At[:], Bt[:])
                nc.vector.tensor_add(Cp[:], At[:], Ht[:])

                Cv = Cp[:].rearrange("p (k t) -> p k t", k=nk)
                for cdim in range(3):
                    nc.vector.tensor_mul(f4[:, k0:k1, cdim],
                                         dif4[:, k0:k1, cdim], Cv)

            # dpdt: with pair orientation (0,1),(2,0),(1,2):
            #   b0 = f0 - f1, b1 = f2 - f0, b2 = f1 - f2
            # Each body-block DMAs out as soon as its subtraction lands, so
            # the store drains while the remaining combines still run.
            opb = op_t[:].rearrange("p (b w) -> p b w", b=3)
            f3 = fv[:].rearrange("p (k w) -> p k w", k=3)
            opr3 = opr[ci].rearrange("p (b w) -> p b w", b=3)
            nc.vector.tensor_sub(opb[:, 0], f3[:, 0], f3[:, 1])
            nc.scalar.dma_start(opr3[:, 0], opb[:, 0])
            nc.vector.tensor_sub(opb[:, 1], f3[:, 2], f3[:, 0])
            nc.scalar.dma_start(opr3[:, 1], opb[:, 1])
            nc.vector.tensor_sub(opb[:, 2], f3[:, 1], f3[:, 2])
            nc.scalar.dma_start(opr3[:, 2], opb[:, 2])

            if general:
                zpb = zp[:].rearrange("p (b w) -> p b w", b=3)
                for b in range(3):
                    nc.scalar.activation(zpb[:, b], zpb[:, b], AF.Copy,
                                         scale=minv_t[:, b:b + 1])
                nc.scalar.dma_start(oqr[ci], zp[:])


_MODULE_CACHE = {}


def _get_module(coefs, qs, nchunk=NCHUNK, Tc=TC, fast=True, streams=None):
    key = (tuple(np.round(coefs, 12)), qs, nchunk, Tc, fast,
           tuple(streams or ()))
    if key not in _MODULE_CACHE:
        nc = _Bacc("TRN2", target_bir_lowering=False, debug=False,
                   num_devices=NCORES)
        rows = nchunk * P
        zq = nc.dram_tensor("zq", [rows, 9 * Tc], I16, kind="ExternalInput").ap()
        outp = nc.dram_tensor("outp", [rows, 9 * Tc], F16,
                              kind="ExternalOutput").ap()
        kw = {}
        if not fast:
            kw["zp_ap"] = nc.dram_tensor("zp", [rows, 9 * Tc], F16,
                                         kind="ExternalInput").ap()
            kw["minv_ap"] = nc.dram_tensor("minv", [P, 3], F32,
                                           kind="ExternalInput").ap()
            kw["outq_ap"] = nc.dram_tensor("outq", [rows, 9 * Tc], F16,
                                           kind="ExternalOutput").ap()
        with tile.TileContext(nc) as tc:
            _emit(tc, zq, outp, coefs, qs, nchunk, Tc, streams=streams, **kw)
        nc.compile()
        _MODULE_CACHE[key] = nc
    return _MODULE_CACHE[key]


def _marshal(arr, nchunk, Tc):
    """[B_core rows, 9] -> [nchunk*P, 9*Tc] field-major per chunk."""
    return np.ascontiguousarray(
        arr.reshape(nchunk, P, Tc, 9).transpose(0, 1, 3, 2)
        .reshape(nchunk * P, 9 * Tc))


def _unmarshal(arr, nchunk, Tc):
    return arr.reshape(nchunk, P, 9, Tc).transpose(0, 1, 3, 2).reshape(-1, 9)


def host_inputs(z, log_m_body, W1, b1, W2, b2, W3, nchunk=NCHUNK, Tc=TC,
                deg=DEG):
    """Returns (in_maps, coefs, qs, fast, B_pad)."""
    z = np.asarray(z, np.float32)
    B = z.shape[0]
    grain = NCORES * P * Tc * nchunk
    B_pad = ((B + grain - 1) // grain) * grain
    if B_pad != B:
        zp_ = np.zeros((B_pad, 18), np.float32)
        zp_[:B] = z
        z = zp_

    coefs = _fit_force_poly_s(W1, b1, W2, b2, W3, deg=deg)
    minv = (np.float64(1.0)
            / (np.exp(np.asarray(log_m_body, np.float64)) + 1e-8))
    fast = bool(np.abs(minv - 1.0).max() < 1e-6)
    minv128 = np.ascontiguousarray(
        np.tile(minv[None, :].astype(np.float32), (P, 1)))

    # adaptive fixed-point scale: largest power of 2 with max|dif|*QS < 32700
    q = z[:, :9]
    qb = q.reshape(-1, 3, 3)
    difmax = max(
        float(np.abs(qb[:, i] - qb[:, j]).max()) for i, j in PAIRS)
    qs = float(2.0 ** np.floor(np.log2(32700.0 / max(difmax, 1e-3))))
    qs = float(min(max(qs, 256.0), 16384.0))

    qi = np.clip(np.round(q.astype(np.float64) * qs),
                 -32767, 32767).astype(np.int16)
    p16 = z[:, 9:].astype(np.float16)

    B_core = B_pad // NCORES
    in_maps = []
    for i in range(NCORES):
        sl = slice(i * B_core, (i + 1) * B_core)
        m = {"zq": _marshal(qi[sl], nchunk, Tc)}
        if not fast:
            m["zp"] = _marshal(p16[sl], nchunk, Tc)
            m["minv"] = minv128
        in_maps.append(m)
    return in_maps, coefs, qs, fast, B_pad


def kernel(z, log_m_body, W1, b1, W2, b2, W3, b3, **_unused):
    z = np.asarray(z, np.float32)
    B = z.shape[0]
    in_maps, coefs, qs, fast, B_pad = host_inputs(
        z, log_m_body, W1, b1, W2, b2, W3)
    nc = _get_module(coefs, qs, fast=fast)
    res = run_bass_kernel_spmd(nc, in_maps, core_ids=list(range(NCORES)))
    dps = [_unmarshal(np.asarray(r["outp"]), NCHUNK, TC) for r in res.results]
    dpdt = np.concatenate(dps, axis=0).astype(np.float32)[:B]
    out = np.empty((B, 18), np.float32)
    if fast:
        # minv == 1 to <=1e-6: dqdt = p exactly (error 5 orders below budget)
        out[:, :9] = z[:B, 9:]
    else:
        dqs = [_unmarshal(np.asarray(r["outq"]), NCHUNK, TC)
               for r in res.results]
        out[:, :9] = np.concatenate(dqs, axis=0).astype(np.float32)[:B]
    out[:, 9:] = dpdt
    return out
